# revision 30
# baseline (speedup 1.0000x reference)
"""CenterNet-style decode for Trainium2, batch-parallel over 8 NeuronCores.

kernel(heat[16,80,128,128], wh, reg, K=100) -> [16,100,6] f32, bit-exact vs
the jax reference (ties broken by lowest flat index, as jax top_k).

Per batch (2 per core): strip-wise 3x3 SAME max-pool NMS in a [class=80
partitions, h*w] layout; per-class top-8 via vector.max + max_index (the
global top-100 provably lies in that union unless >=9 of the top-100 share
one class -- guarded by a flag); 13 rounds of max/max_index/match_replace
over the 640-candidate union extract the top-104 with exact jax tie
semantics; winner metadata (spatial idx, wh, reg) is fetched with
per-partition-offset indirect DMAs in a [winner=partition] column layout
(this runtime only supports one offset per partition, contiguous run).
All partition-reshaping data movement bounces through DRAM scratch
(SBUF->SBUF partition-reshape descriptors fail to load here).
"""

import sys

sys.path.insert(0, "/opt/trn_rl_repo")

import numpy as np

import bass_rust
import concourse.bass as bass
import concourse.tile as tile
from concourse import mybir
from concourse.vector_clock import ScopedClock

B, C, H, W = 16, 80, 128, 128
HW = H * W
K = 100
NCORES = 8
BPC = B // NCORES
NSTRIP = 8
SH = H // NSTRIP
KPAD = 104
NU = C * 8
NEG = -1.0e30
F32 = mybir.dt.float32
U32 = mybir.dt.uint32
ALU = mybir.AluOpType


def _split_excess_waits(nc):
    """This walrus build accepts at most ONE sync wait per instruction.
    Hoist excess waits onto same-engine NoOps inserted just before."""
    for fn in nc.m.functions:
        for bb in fn.blocks:
            new_insts = []
            for inst in bb.instructions:
                si = inst.sync_info
                waits = list(si.on_wait) if (si is not None and si.on_wait) else []
                if len(waits) > 1:
                    si.on_wait = waits[:1]
                    for w in waits[1:]:
                        nop = mybir.InstNoOp(
                            name=nc.get_next_instruction_name(),
                            ins=[],
                            outs=[],
                            hint="waitsplit",
                        )
                        nop.engine = inst.engine
                        nop.sync_info = bass_rust.SyncInfo(on_wait=[w], on_update=[])
                        nc.register_instruction(nop, overwrite=True)
                        new_insts.append(nop)
                new_insts.append(inst)
            bb.instructions[:] = new_insts


def _patched_drain_and_barrier(self, tick_clock, wait_clock):
    nc = self.nc
    drain_inst = nc.sync.drain()
    wait_clock.add_sem_waits(
        drain_inst.ins, ScopedClock({None: tick_clock.global_clock})
    )
    si = drain_inst.ins.sync_info
    waits = list(si.on_wait or []) if si is not None else []
    if waits:
        si.on_wait = []
        for i, w in enumerate(waits):
            n = nc.sync.nop(hint=f"waitsplit{i}", nofuse=True)
            n.ins.sync_info = bass_rust.SyncInfo(on_wait=[w], on_update=[])
    nc.all_engine_barrier()
    assert self.sems is not None
    popped = nc._tile_sem_poison_stack.pop()
    assert popped is self._sem_poison
    nc.clear_and_free_semaphores(list(self.sems.allocated().values()))
    nc.all_engine_barrier()
    _split_excess_waits(nc)


tile.TileContext._drain_and_barrier = _patched_drain_and_barrier


def build_program():
    nc = bass.Bass("TRN2", target_bir_lowering=False, debug=False)

    heat = nc.dram_tensor("heat", [BPC, C, H, W], F32, kind="ExternalInput").ap()
    wh = nc.dram_tensor("wh", [BPC, 2, H, W], F32, kind="ExternalInput").ap()
    reg = nc.dram_tensor("reg", [BPC, 2, H, W], F32, kind="ExternalInput").ap()
    out = nc.dram_tensor("out", [BPC, K, 6], F32, kind="ExternalOutput").ap()
    flags = nc.dram_tensor("flags", [BPC, 1], F32, kind="ExternalOutput").ap()
    scr = {
        "fl_v": nc.dram_tensor("fl_v", [BPC, NU], F32).ap(),
        "fl_i": nc.dram_tensor("fl_i", [BPC, NU], U32).ap(),
        "fl_g": nc.dram_tensor("fl_g", [BPC, C], F32).ap(),
        "xig": nc.dram_tensor("xig_scr", [BPC, KPAD], U32).ap(),
        "sco": nc.dram_tensor("sco_scr", [BPC, KPAD], F32).ap(),
        "wr": nc.dram_tensor("wr_scr", [1, BPC * HW * 4], F32).ap(),
    }

    with tile.TileContext(nc) as tc:
        build_tile_kernel(tc, heat, wh, reg, out, flags, scr)
    return nc


def build_tile_kernel(tc, heat, wh, reg, out, flags, scr):
    from contextlib import ExitStack

    nc = tc.nc
    ctx = ExitStack()
    with ctx:
        nms_pool = ctx.enter_context(tc.tile_pool(name="nms", bufs=1))
        xs_pool = ctx.enter_context(tc.tile_pool(name="xstrip", bufs=2))
        t_pool = ctx.enter_context(tc.tile_pool(name="tstrip", bufs=2))
        u_pool = ctx.enter_context(tc.tile_pool(name="ustrip", bufs=1))
        cand_pool = ctx.enter_context(tc.tile_pool(name="cand", bufs=1))
        sp = ctx.enter_context(tc.tile_pool(name="small", bufs=1))

        heat3 = heat  # [BPC, C, H, W]

        # ---- strip-wise NMS into per-batch nms buffers ------------------
        V, I = [], []
        for b in range(BPC):
            nmsb = nms_pool.tile([C, HW], F32, tag=f"nms{b}")
            nms3 = nmsb[:].rearrange("c (h w) -> c h w", w=W)
            for s in range(NSTRIP):
                h0, h1 = s * SH, (s + 1) * SH
                lo = max(h0 - 1, 0)
                hi = min(h1 + 1, H)
                hh = hi - lo
                off = h0 - lo
                xs = xs_pool.tile([C, (SH + 2) * W], F32)
                x3 = xs[:, : hh * W].rearrange("c (h w) -> c h w", w=W)
                nc.sync.dma_start(x3, heat3[b, :, lo:hi, :])
                a = x3[:, off : off + SH, :]
                t = t_pool.tile([C, SH * W], F32)
                t3 = t[:].rearrange("c (h w) -> c h w", w=W)
                if s < NSTRIP - 1:
                    nc.vector.tensor_tensor(
                        out=t3, in0=a, in1=x3[:, off + 1 : off + SH + 1, :], op=ALU.max
                    )
                else:
                    nc.vector.tensor_tensor(
                        out=t3[:, : SH - 1, :],
                        in0=a[:, : SH - 1, :],
                        in1=x3[:, off + 1 : off + SH, :],
                        op=ALU.max,
                    )
                    nc.scalar.mul(t3[:, SH - 1 :, :], a[:, SH - 1 :, :], 1.0)
                if s > 0:
                    nc.vector.tensor_tensor(
                        out=t3, in0=t3, in1=x3[:, off - 1 : off + SH - 1, :], op=ALU.max
                    )
                else:
                    nc.vector.tensor_tensor(
                        out=t3[:, 1:, :],
                        in0=t3[:, 1:, :],
                        in1=x3[:, 0 : SH - 1, :],
                        op=ALU.max,
                    )
                u = u_pool.tile([C, SH * W], F32)
                u3 = u[:].rearrange("c (h w) -> c h w", w=W)
                nc.vector.tensor_tensor(
                    out=u3[:, :, : W - 1],
                    in0=t3[:, :, : W - 1],
                    in1=t3[:, :, 1:],
                    op=ALU.max,
                )
                nc.scalar.mul(u3[:, :, W - 1 :], t3[:, :, W - 1 :], 1.0)
                nc.vector.tensor_tensor(
                    out=u3[:, :, 1:],
                    in0=u3[:, :, 1:],
                    in1=t3[:, :, : W - 1],
                    op=ALU.max,
                )
                nc.vector.tensor_tensor(out=u3, in0=u3, in1=a, op=ALU.is_equal)
                nc.vector.tensor_tensor(
                    out=nms3[:, h0:h1, :], in0=u3, in1=a, op=ALU.mult
                )

            vb = cand_pool.tile([C, 8], F32, tag=f"v{b}")
            ib = cand_pool.tile([C, 8], U32, tag=f"i{b}")
            nc.vector.max(out=vb[:], in_=nmsb[:])
            nc.vector.max_index(out=ib[:], in_max=vb[:], in_values=nmsb[:])
            V.append(vb)
            I.append(ib)
            # flatten candidates via DRAM (SBUF->SBUF partition reshapes
            # don't load on this runtime)
            nc.sync.dma_start(
                scr["fl_v"][b].rearrange("(c k) -> c k", k=8), V[b][:]
            )
            nc.sync.dma_start(
                scr["fl_i"][b].rearrange("(c k) -> c k", k=8), I[b][:]
            )
            nc.sync.dma_start(
                scr["fl_g"][b].rearrange("(c k) -> c k", k=1), V[b][:, 7:8]
            )

        uv = sp.tile([BPC, NU], F32, tag="uv")
        g8 = sp.tile([BPC, C], F32, tag="g8")
        nc.sync.dma_start(uv[:], scr["fl_v"][:, :])
        nc.sync.dma_start(g8[:], scr["fl_g"][:, :])

        # ---- extraction: top-104, ties by (value desc, position asc)
        S = sp.tile([BPC, KPAD], F32, tag="scores")
        XI = sp.tile([BPC, KPAD], U32, tag="xi")
        for j in range(13):
            sj = S[:, 8 * j : 8 * j + 8]
            nc.vector.max(out=sj, in_=uv[:])
            nc.vector.max_index(
                out=XI[:, 8 * j : 8 * j + 8], in_max=sj, in_values=uv[:]
            )
            if j < 12:
                nc.vector.match_replace(
                    out=uv[:], in_to_replace=sj, in_values=uv[:], imm_value=NEG
                )

        # ---- guard
        gmax = sp.tile([BPC, 1], F32, tag="gmax")
        nc.vector.tensor_reduce(
            out=gmax[:], in_=g8[:], axis=mybir.AxisListType.X, op=ALU.max
        )
        flg = sp.tile([BPC, 1], F32, tag="flg")
        nc.vector.tensor_tensor(
            out=flg[:], in0=gmax[:], in1=S[:, K - 1 : K], op=ALU.is_ge
        )
        nc.sync.dma_start(flags[:, :], flg[:])

        # ---- winner positions within the 640-union, to DRAM for the tail
        nc.sync.dma_start(scr["xig"][:, :], XI[:])
        nc.sync.dma_start(scr["sco"][:, :], S[:])

        # ---- per-batch column-layout tail: winner = partition ------------
        fl_i_flat = scr["fl_i"].rearrange("(o b) n -> o (b n)", o=1)
        wh_flat = wh.rearrange("b c h w -> (b c) (h w)")
        reg_flat = reg.rearrange("b c h w -> (b c) (h w)")
        for b in range(BPC):
            xcol = sp.tile([KPAD, 1], U32, tag=f"xcol{b}")
            nc.sync.dma_start(
                xcol[:], scr["xig"][b, :].rearrange("(k o) -> k o", o=1)
            )
            scol = sp.tile([KPAD, 1], F32, tag=f"scol{b}")
            nc.sync.dma_start(
                scol[:], scr["sco"][b, :].rearrange("(k o) -> k o", o=1)
            )
            # class = pos//8 ; global union offset for the gather = pos + b*NU
            cls_u = sp.tile([KPAD, 1], U32, tag=f"clsu{b}")
            nc.vector.tensor_scalar(
                out=cls_u[:], in0=xcol[:], scalar1=3, scalar2=None,
                op0=ALU.logical_shift_right,
            )
            cls_f = sp.tile([KPAD, 1], F32, tag=f"clsf{b}")
            nc.vector.tensor_copy(out=cls_f[:], in_=cls_u[:])
            bcNU = sp.tile([KPAD, 1], U32, tag=f"bcNU{b}")
            nc.vector.memset(bcNU[:], b * NU)
            nc.vector.tensor_tensor(
                out=xcol[:], in0=xcol[:], in1=bcNU[:], op=ALU.add
            )
            # spatial index: one gather, per-partition offset, run of 1
            s_u = sp.tile([KPAD, 1], U32, tag=f"su{b}")
            nc.gpsimd.indirect_dma_start(
                out=s_u[:],
                out_offset=None,
                in_=fl_i_flat,
                in_offset=bass.IndirectOffsetOnAxis(ap=xcol[:], axis=1),
            )
            ys_u = sp.tile([KPAD, 1], U32, tag=f"ysu{b}")
            xs_u = sp.tile([KPAD, 1], U32, tag=f"xsu{b}")
            nc.vector.tensor_scalar(
                out=ys_u[:], in0=s_u[:], scalar1=7, scalar2=None,
                op0=ALU.logical_shift_right,
            )
            nc.vector.tensor_scalar(
                out=xs_u[:], in0=s_u[:], scalar1=127, scalar2=None,
                op0=ALU.bitwise_and,
            )
            ys_f = sp.tile([KPAD, 1], F32, tag=f"ysf{b}")
            xs_f = sp.tile([KPAD, 1], F32, tag=f"xsf{b}")
            nc.vector.tensor_copy(out=ys_f[:], in_=ys_u[:])
            nc.vector.tensor_copy(out=xs_f[:], in_=xs_u[:])
            # wh/reg: 4 per-plane gathers at offset b*2HW + ch*HW + s
            wrg = sp.tile([KPAD, 4], F32, tag=f"wrg{b}")
            offp = sp.tile([KPAD, 1], U32, tag=f"offp{b}")
            bhw = sp.tile([KPAD, 1], U32, tag=f"bhw{b}")
            nc.vector.memset(bhw[:], b * 2 * HW)
            nc.vector.tensor_tensor(out=offp[:], in0=s_u[:], in1=bhw[:], op=ALU.add)
            hwc = sp.tile([KPAD, 1], U32, tag=f"hwc{b}")
            nc.vector.memset(hwc[:], HW)
            for comp, srct in ((0, wh_flat), (1, wh_flat), (2, reg_flat), (3, reg_flat)):
                if comp in (1, 3):
                    nc.vector.tensor_tensor(
                        out=offp[:], in0=offp[:], in1=hwc[:], op=ALU.add
                    )
                if comp == 2:
                    nc.vector.tensor_tensor(
                        out=offp[:], in0=offp[:], in1=hwc[:], op=ALU.subtract
                    )
                nc.gpsimd.indirect_dma_start(
                    out=wrg[:, comp : comp + 1],
                    out_offset=None,
                    in_=srct,
                    in_offset=bass.IndirectOffsetOnAxis(ap=offp[:], axis=1),
                )
            # assemble [K, 6] = x1 y1 x2 y2 score class
            kk = slice(0, K)
            xc = sp.tile([KPAD, 1], F32, tag=f"xc{b}")
            yc = sp.tile([KPAD, 1], F32, tag=f"yc{b}")
            h0t = sp.tile([KPAD, 1], F32, tag=f"h0t{b}")
            h1t = sp.tile([KPAD, 1], F32, tag=f"h1t{b}")
            nc.vector.tensor_tensor(
                out=xc[:], in0=xs_f[:], in1=wrg[:, 2:3], op=ALU.add
            )
            nc.vector.tensor_tensor(
                out=yc[:], in0=ys_f[:], in1=wrg[:, 3:4], op=ALU.add
            )
            nc.vector.tensor_scalar_mul(h0t[:], wrg[:, 0:1], 0.5)
            nc.vector.tensor_scalar_mul(h1t[:], wrg[:, 1:2], 0.5)
            ob = sp.tile([KPAD, 6], F32, tag=f"ob{b}")
            nc.vector.tensor_tensor(
                out=ob[:, 0:1], in0=xc[:], in1=h0t[:], op=ALU.subtract
            )
            nc.vector.tensor_tensor(
                out=ob[:, 1:2], in0=yc[:], in1=h1t[:], op=ALU.subtract
            )
            nc.vector.tensor_tensor(out=ob[:, 2:3], in0=xc[:], in1=h0t[:], op=ALU.add)
            nc.vector.tensor_tensor(out=ob[:, 3:4], in0=yc[:], in1=h1t[:], op=ALU.add)
            nc.vector.tensor_copy(out=ob[:, 4:5], in_=scol[:])
            nc.vector.tensor_copy(out=ob[:, 5:6], in_=cls_f[:])
            nc.sync.dma_start(out[b], ob[kk, :])


_NC_CACHE = {}


def _get_program():
    if "nc" not in _NC_CACHE:
        _NC_CACHE["nc"] = build_program()
    return _NC_CACHE["nc"]


def kernel(heat, wh, reg, K):
    assert int(K) == 100
    heat = np.ascontiguousarray(np.asarray(heat, dtype=np.float32))
    wh = np.ascontiguousarray(np.asarray(wh, dtype=np.float32))
    reg = np.ascontiguousarray(np.asarray(reg, dtype=np.float32))
    assert heat.shape == (B, C, H, W)

    nc = _get_program()
    in_maps = []
    for i in range(NCORES):
        sl = slice(i * BPC, (i + 1) * BPC)
        in_maps.append(
            {
                "heat": np.ascontiguousarray(heat[sl]),
                "wh": np.ascontiguousarray(wh[sl]),
                "reg": np.ascontiguousarray(reg[sl]),
            }
        )
    from concourse.bass_utils import run_bass_kernel_spmd

    res = run_bass_kernel_spmd(nc, in_maps, list(range(NCORES)))
    outs = []
    for i in range(NCORES):
        r = res.results[i]
        if np.any(r["flags"] != 0.0):
            raise RuntimeError(f"top-k guard tripped on core {i}")
        outs.append(r["out"])
    return np.concatenate(outs, axis=0)
